# revision 5
# baseline (speedup 1.0000x reference)
"""Trainium2 Bass kernel for the LIF-network step (nn_NetworkClass_31018253812098).

Computation (reference, all fp32, N = NN = N_IN = 2048):
    z_out_new = BETA * z_out + z
    v_new     = ALPHA * v + x @ w - V_TH * z + z_out_new @ wrec
    mask      = (v_new[0, :] - V_TH) > 0          # length-2048, from batch row 0
    z_new[i, j] = mask[i]                         # row-broadcast (N == NN)

Device strategy: the whole problem is ONE fused GEMM,
    S = [x | zc] @ [[w], [wrec]]                  # contraction 4096
with everything else O(N^2), which the host does exactly in fp32: z_out_new,
alpha*v - v_th*z, the rank-1 mean-correction mu*colsum(wrec), the row-0 mask
matvec (fp64), and the z_new broadcast.

Dtypes: x/w in bf16 (1 col/cycle); zc = z_out_new - mean in e4m3 fp8 with
wrec in fp8 via DoubleRow perf mode (2 contraction rows/cycle).  Mean-centering
zon (a uniform-positive tensor) cuts its fp8 quantization error ~2.3x; the
removed rank-1 term is restored exactly on the host.  Measured end-to-end
v_new rel err 7.2e-3 vs the 2e-2 gate; the mask is host-exact so threshold
flips are impossible.

Sharding: 4x2 grid -- 4 batch shards (512 moving cols, transposed domain) x 2
feature halves (1024 out rows).  Per-core: at_x 2.1 MB bf16 + at_z 1.05 MB fp8
+ w-half 4.2 MB bf16 + wrec-half 2.1 MB fp8 + S-out 1.05 MB bf16 = 10.5 MB
(~25 us DMA) vs PE 27.5 us (MM1) + 7-14 us (MM2 DoubleRow) -> PE-bound.

Startup: DMA issues are spread across the sync/vector/scalar queues so they
parallelize, and the first weight/activation pieces are small (0.25 MB) so the
first matmul fires ~6 us earlier than with 1 MiB chunks.  All tiles are
resident (80 KiB/partition) -- no pool recycling dependencies.
"""

import sys

sys.path.insert(0, "/opt/trn_rl_repo")

import numpy as np
import ml_dtypes

import concourse.mybir as mybir
import concourse.tile as tile
from concourse import bacc, bass_utils

N = 2048
P = 128
KTX = 16             # k-tiles per operand stream (2048 / 128)
NCORES = 8
R, C = 4, 2          # batch shards x feature halves
MS = N // R          # 512 moving (batch) cols per core
NH = N // C          # 1024 out features per core
ALPHA = 1.0 - 0.05 / 10.0   # 0.995
BETA = 1.0 - 0.05 / 2.0     # 0.975
V_TH = 2.0

F32 = mybir.dt.float32
BF16 = mybir.dt.bfloat16
F8 = mybir.dt.float8e4
BF16_NP = ml_dtypes.bfloat16
F8_NP = ml_dtypes.float8_e4m3
DR = mybir.MatmulPerfMode.DoubleRow

# k-tile splits of the early DMAs: small first pieces so the PE starts early
WX_PIECES = [(0, 2), (2, 5), (5, 8), (8, 16)]
ATX_PIECES = [(0, 2), (2, 5), (5, 10), (10, 16)]


def _build_program():
    # bacc (not raw Bass): its compile pass splits multi-semaphore sync
    # waits that walrus's per-instruction wait limit rejects.
    nc = bacc.Bacc("TRN2", target_bir_lowering=False, debug=False, num_devices=NCORES)

    atx = nc.dram_tensor("atx", [P, KTX, MS], BF16, kind="ExternalInput").ap()
    atz = nc.dram_tensor("atz", [P, KTX, MS], F8, kind="ExternalInput").ap()
    # [q, kc, p, a, n]: chunk-major, row = kc*1024 + a*128 + p, col = q*512 + n
    wx = nc.dram_tensor("wx", [2, 2, P, 8, MS], BF16, kind="ExternalInput").ap()
    # [q, p, k, n]: row = k*128 + p, col = q*512 + n
    wr = nc.dram_tensor("wr", [2, P, KTX, MS], F8, kind="ExternalInput").ap()
    sout = nc.dram_tensor("sout", [P, 8, MS], BF16, kind="ExternalOutput").ap()

    with tile.TileContext(nc) as tc:
        with (
            tc.tile_pool(name="res", bufs=1) as res,
            tc.tile_pool(name="psum", bufs=8, space="PSUM") as psum_pool,
        ):
            atx_s = res.tile([P, KTX, MS], BF16, tag="atx_s")
            atz_s = res.tile([P, KTX, MS], F8, tag="atz_s")
            wx_s = [res.tile([P, KTX, MS], BF16, tag="wx", name=f"wx{q}") for q in range(2)]
            wr_s = [res.tile([P, KTX, MS], F8, tag="wr", name=f"wr{q}") for q in range(2)]
            st_s = res.tile([P, 8, MS], BF16, tag="st_s")

            # --- DMA issue: emission order per engine = queue priority.
            # sync: weights (the big stream); vector: at_x; scalar: at_z.
            for a0, a1 in WX_PIECES:
                kc = a0 // 8
                nc.sync.dma_start(
                    wx_s[0][:, a0:a1, :], wx[0, kc, :, a0 - 8 * kc : a1 - 8 * kc, :]
                )
            nc.sync.dma_start(wr_s[0][:, 0:8, :], wr[0, :, 0:8, :])
            nc.sync.dma_start(wr_s[0][:, 8:16, :], wr[0, :, 8:16, :])
            nc.sync.dma_start(wx_s[1][:, 0:8, :], wx[1, 0])
            nc.sync.dma_start(wx_s[1][:, 8:16, :], wx[1, 1])
            nc.sync.dma_start(wr_s[1][:, 0:8, :], wr[1, :, 0:8, :])
            nc.sync.dma_start(wr_s[1][:, 8:16, :], wr[1, :, 8:16, :])
            for a0, a1 in ATX_PIECES:
                nc.gpsimd.dma_start(atx_s[:, a0:a1, :], atx[:, a0:a1, :])
            nc.scalar.dma_start(atz_s[:, 0:8, :], atz[:, 0:8, :])
            nc.scalar.dma_start(atz_s[:, 8:16, :], atz[:, 8:16, :])

            for q in range(2):
                ps = [
                    psum_pool.tile([P, MS], F32, tag="ps", name=f"ps{q}_{i}")
                    for i in range(4)
                ]
                # MM1: x @ w in bf16, k-tiles 0..15
                for k in range(KTX):
                    for n in range(4):
                        nc.tensor.matmul(
                            ps[n][:],
                            lhsT=wx_s[q][:, k, n * P : (n + 1) * P],
                            rhs=atx_s[:, k, :],
                            start=(k == 0),
                            stop=False,
                        )
                # MM2: zc @ wrec in fp8 DoubleRow (k-pairs), first 4 pairs
                for j in range(4):
                    for n in range(4):
                        nc.tensor.matmul(
                            ps[n][:],
                            lhsT=wr_s[q][:, 2 * j : 2 * j + 2, n * P : (n + 1) * P],
                            rhs=atz_s[:, 2 * j : 2 * j + 2, :],
                            start=False,
                            stop=False,
                            perf_mode=DR,
                        )
                # last 4 pairs n-major so psum banks finish staggered and the
                # drain (copy + store) hides under the remaining matmuls
                for n in range(4):
                    for j in range(4, 8):
                        nc.tensor.matmul(
                            ps[n][:],
                            lhsT=wr_s[q][:, 2 * j : 2 * j + 2, n * P : (n + 1) * P],
                            rhs=atz_s[:, 2 * j : 2 * j + 2, :],
                            start=False,
                            stop=(j == 7),
                            perf_mode=DR,
                        )
                    t = q * 4 + n
                    nc.scalar.copy(st_s[:, t, :], ps[n][:])
                    nc.gpsimd.dma_start(sout[:, t, :], st_s[:, t, :])

    nc.compile()
    return nc


_PROGRAM_CACHE = {}


def _get_program():
    if "nc" not in _PROGRAM_CACHE:
        _PROGRAM_CACHE["nc"] = _build_program()
    return _PROGRAM_CACHE["nc"]


def make_in_maps(x, zc, w, wrec):
    """x fp32 [2048,2048]; zc fp32 centered zon; w/wrec fp32."""
    xT = np.ascontiguousarray(x.T).astype(BF16_NP)
    zcT = np.ascontiguousarray(zc.T).astype(F8_NP)

    wx_packed, wr_packed = [], []
    for nh in range(C):
        cols = slice(nh * NH, (nh + 1) * NH)
        wh = w[:, cols].astype(BF16_NP)
        t = wh.reshape(2, 8, P, 2, MS)  # [kc, a, p, q, n]
        wx_packed.append(np.ascontiguousarray(t.transpose(3, 0, 2, 1, 4)))
        wrh = wrec[:, cols].astype(F8_NP)
        t = wrh.reshape(KTX, P, 2, MS)  # [k, p, q, n]
        wr_packed.append(np.ascontiguousarray(t.transpose(2, 1, 0, 3)))

    in_maps = []
    for c in range(NCORES):
        nh, ms = divmod(c, R)
        mc = slice(ms * MS, (ms + 1) * MS)
        ax = np.ascontiguousarray(
            xT[:, mc].reshape(KTX, P, MS).transpose(1, 0, 2)
        )
        az = np.ascontiguousarray(
            zcT[:, mc].reshape(KTX, P, MS).transpose(1, 0, 2)
        )
        in_maps.append(
            {"atx": ax, "atz": az, "wx": wx_packed[nh], "wr": wr_packed[nh]}
        )
    return in_maps


def kernel(x, v, z, z_out, w, wrec, _trace=False):
    x = np.ascontiguousarray(x, dtype=np.float32)
    v = np.ascontiguousarray(v, dtype=np.float32)
    z = np.ascontiguousarray(z, dtype=np.float32)
    z_out = np.ascontiguousarray(z_out, dtype=np.float32)
    w = np.ascontiguousarray(w, dtype=np.float32)
    wrec = np.ascontiguousarray(wrec, dtype=np.float32)

    z_out_new = BETA * z_out + z  # exact fp32; also the GEMM's second operand
    mu = np.float32(z_out_new.mean(dtype=np.float64))
    zc = z_out_new - mu

    nc = _get_program()
    in_maps = make_in_maps(x, zc, w, wrec)
    res = bass_utils.run_bass_kernel_spmd(
        nc, in_maps, core_ids=list(range(NCORES)), trace=_trace
    )

    S = np.empty((N, N), np.float32)
    for c, r in enumerate(res.results):
        nh, ms = divmod(c, R)
        blk = r["sout"].astype(np.float32).transpose(1, 0, 2).reshape(NH, MS)
        S[ms * MS : (ms + 1) * MS, nh * NH : (nh + 1) * NH] = blk.T

    # restore the rank-1 term removed by mean-centering: mu * colsum(wrec)
    colsum = wrec.sum(axis=0, dtype=np.float64).astype(np.float32)
    v_new = ALPHA * v - V_TH * z + S + mu * colsum[None, :]
    # batch row 0 drives the threshold mask: recompute it exactly on host
    # (fp64 matvecs) so GEMM quantization noise can never flip a mask bit.
    row0 = (
        ALPHA * v[0].astype(np.float64)
        - V_TH * z[0].astype(np.float64)
        + x[0].astype(np.float64) @ w.astype(np.float64)
        + z_out_new[0].astype(np.float64) @ wrec.astype(np.float64)
    )
    v_new[0, :] = row0.astype(np.float32)
    mask = (v_new[0, :] - V_TH) > 0.0
    z_new = np.ascontiguousarray(
        np.broadcast_to(mask[:, None].astype(np.float32), (N, N))
    )
    out = (v_new, z_new, z_out_new)
    if _trace:
        return out, res
    return out


# revision 9
# speedup vs baseline: 1.1847x; 1.1847x over previous
"""Trainium2 Bass kernel for the LIF-network step (nn_NetworkClass_31018253812098).

Computation (reference, all fp32, N = NN = N_IN = 2048):
    z_out_new = BETA * z_out + z
    v_new     = ALPHA * v + x @ w - V_TH * z + z_out_new @ wrec
    mask      = (v_new[0, :] - V_TH) > 0          # length-2048, from batch row 0
    z_new[i, j] = mask[i]                         # row-broadcast (N == NN)

Device strategy: the whole problem is ONE fused GEMM,
    S = [x | zc] @ [[w], [wrec]]                  # contraction 4096
with everything else O(N^2), which the host does exactly in fp32: z_out_new,
alpha*v - v_th*z, the rank-1 mean-correction mu*colsum(wrec), the row-0 mask
matvec (fp64), and the z_new broadcast.

Dtypes: x/w in bf16 (1 col/cycle); zc = z_out_new - mean in e4m3 fp8 with
wrec in fp8 via DoubleRow perf mode (2 contraction rows/cycle).  Mean-centering
zon (a uniform-positive tensor) cuts its fp8 quantization error ~2.3x; the
removed rank-1 term is restored exactly on the host.  Measured end-to-end
v_new rel err 7.2e-3 vs the 2e-2 gate; the mask is host-exact so threshold
flips are impossible.

Sharding: 4x2 grid -- 4 batch shards (512 moving cols, transposed domain) x 2
feature halves (1024 out rows).  Per-core: at_x 2.1 MB bf16 + at_z 1.05 MB fp8
+ w-half 4.2 MB bf16 + wrec-half 2.1 MB fp8 + S-out 1.05 MB bf16 = 10.5 MB
(~25 us DMA) vs PE 27.5 us (MM1) + 7-14 us (MM2 DoubleRow) -> PE-bound.

Startup: DMA issues are spread across the sync/vector/scalar queues so they
parallelize, and the first weight/activation pieces are small (0.25 MB) so the
first matmul fires ~6 us earlier than with 1 MiB chunks.  All tiles are
resident (80 KiB/partition) -- no pool recycling dependencies.
"""

import sys

sys.path.insert(0, "/opt/trn_rl_repo")

import numpy as np
import ml_dtypes

import concourse.mybir as mybir
import concourse.tile as tile
from concourse import bacc, bass_utils

N = 2048
P = 128
KTX = 16             # k-tiles per operand stream (2048 / 128)
NCORES = 8
R, C = 4, 2          # batch shards x feature halves
MS = N // R          # 512 moving (batch) cols per core
NH = N // C          # 1024 out features per core
ALPHA = 1.0 - 0.05 / 10.0   # 0.995
BETA = 1.0 - 0.05 / 2.0     # 0.975
V_TH = 2.0

F32 = mybir.dt.float32
BF16 = mybir.dt.bfloat16
F8 = mybir.dt.float8e4
BF16_NP = ml_dtypes.bfloat16
F8_NP = ml_dtypes.float8_e4m3
DR = mybir.MatmulPerfMode.DoubleRow

# k-tile splits of the early DMAs: 8-k-tile pieces keep 8 KiB descriptor rows
# (smaller rows are descriptor-rate-bound on the cold DMA engines and arrive
# LATER than a full 1 MiB piece)
WX_PIECES = [(0, 8), (8, 16)]
ATX_PIECES = [(0, 8), (8, 16)]
NWARM = 48           # PE-warmup dummy matmuls (96 cols) to finish the p-state
                     # ramp while the first input DMAs are still in flight


def _build_program():
    # bacc (not raw Bass): its compile pass splits multi-semaphore sync
    # waits that walrus's per-instruction wait limit rejects.
    nc = bacc.Bacc("TRN2", target_bir_lowering=False, debug=False, num_devices=NCORES)

    atx = nc.dram_tensor("atx", [P, KTX, MS], BF16, kind="ExternalInput").ap()
    atz = nc.dram_tensor("atz", [P, KTX, MS], F8, kind="ExternalInput").ap()
    # [q, kc, p, a, n]: chunk-major, row = kc*1024 + a*128 + p, col = q*512 + n
    wx = nc.dram_tensor("wx", [2, 2, P, 8, MS], BF16, kind="ExternalInput").ap()
    # [q, p, k, n]: row = k*128 + p, col = q*512 + n
    wr = nc.dram_tensor("wr", [2, P, KTX, MS], F8, kind="ExternalInput").ap()
    sout = nc.dram_tensor("sout", [P, 8, MS], BF16, kind="ExternalOutput").ap()

    with tile.TileContext(nc) as tc:
        with (
            tc.tile_pool(name="res", bufs=1) as res,
            tc.tile_pool(name="psum", bufs=8, space="PSUM") as psum_pool,
        ):
            atx_s = res.tile([P, KTX, MS], BF16, tag="atx_s")
            atz_s = res.tile([P, KTX, MS], F8, tag="atz_s")
            wx_s = [res.tile([P, KTX, MS], BF16, tag="wx", name=f"wx{q}") for q in range(2)]
            wr_s = [res.tile([P, KTX, MS], F8, tag="wr", name=f"wr{q}") for q in range(2)]
            st_s = res.tile([P, 8, MS], BF16, tag="st_s")
            warm_s = res.tile([P, P], BF16, tag="warm_s")

            # --- DMA issue: emission order per engine = queue priority.
            # sync: weights (the big stream); scalar: activations; gpsimd is
            # kept DMA-free until the output stores so its end-drain is cheap.
            nc.sync.dma_start(wx_s[0][:, 0:8, :], wx[0, 0])
            nc.sync.dma_start(wx_s[0][:, 8:16, :], wx[0, 1])
            nc.sync.dma_start(wr_s[0][:], wr[0])
            nc.sync.dma_start(wx_s[1][:, 0:8, :], wx[1, 0])
            nc.sync.dma_start(wx_s[1][:, 8:16, :], wx[1, 1])
            nc.sync.dma_start(wr_s[1][:], wr[1])
            for a0, a1 in ATX_PIECES:
                nc.scalar.dma_start(atx_s[:, a0:a1, :], atx[:, a0:a1, :])
            nc.scalar.dma_start(atz_s[:], atz[:])

            ps_all = [
                [
                    psum_pool.tile([P, MS], F32, tag="ps", name=f"ps{q}_{i}")
                    for i in range(4)
                ]
                for q in range(2)
            ]

            # PE p-state warmup: dependency-free dummy matmuls run right after
            # the preamble, so the 0.65->2.4 GHz ramp completes while the
            # first input DMAs are still in flight.  Results land in ps0_0,
            # which the first real (start=True) matmul resets anyway.
            nc.vector.memzero(warm_s[:])
            for i in range(NWARM):
                nc.tensor.matmul(
                    ps_all[0][0][:, 0:96],
                    lhsT=warm_s[:],
                    rhs=warm_s[:, 0:96],
                    start=True,
                    stop=True,
                )

            for q in range(2):
                ps = ps_all[q]
                # MM1: x @ w in bf16, k-tiles 0..15
                for k in range(KTX):
                    for n in range(4):
                        nc.tensor.matmul(
                            ps[n][:],
                            lhsT=wx_s[q][:, k, n * P : (n + 1) * P],
                            rhs=atx_s[:, k, :],
                            start=(k == 0),
                            stop=False,
                        )
                # MM2: zc @ wrec in fp8 DoubleRow (k-pairs), first 4 pairs
                for j in range(4):
                    for n in range(4):
                        nc.tensor.matmul(
                            ps[n][:],
                            lhsT=wr_s[q][:, 2 * j : 2 * j + 2, n * P : (n + 1) * P],
                            rhs=atz_s[:, 2 * j : 2 * j + 2, :],
                            start=False,
                            stop=False,
                            perf_mode=DR,
                        )
                # last 4 pairs n-major so psum banks finish staggered and the
                # drain (copy + store) hides under the remaining matmuls
                for n in range(4):
                    for j in range(4, 8):
                        nc.tensor.matmul(
                            ps[n][:],
                            lhsT=wr_s[q][:, 2 * j : 2 * j + 2, n * P : (n + 1) * P],
                            rhs=atz_s[:, 2 * j : 2 * j + 2, :],
                            start=False,
                            stop=(j == 7),
                            perf_mode=DR,
                        )
                    t = q * 4 + n
                    nc.scalar.copy(st_s[:, t, :], ps[n][:])
                    nc.gpsimd.dma_start(sout[:, t, :], st_s[:, t, :])

    nc.compile()
    return nc


_PROGRAM_CACHE = {}


def _get_program():
    if "nc" not in _PROGRAM_CACHE:
        _PROGRAM_CACHE["nc"] = _build_program()
    return _PROGRAM_CACHE["nc"]


def make_in_maps(x, zc, w, wrec):
    """x fp32 [2048,2048]; zc fp32 centered zon; w/wrec fp32."""
    xT = np.ascontiguousarray(x.T).astype(BF16_NP)
    zcT = np.ascontiguousarray(zc.T).astype(F8_NP)

    wx_packed, wr_packed = [], []
    for nh in range(C):
        cols = slice(nh * NH, (nh + 1) * NH)
        wh = w[:, cols].astype(BF16_NP)
        t = wh.reshape(2, 8, P, 2, MS)  # [kc, a, p, q, n]
        wx_packed.append(np.ascontiguousarray(t.transpose(3, 0, 2, 1, 4)))
        wrh = wrec[:, cols].astype(F8_NP)
        t = wrh.reshape(KTX, P, 2, MS)  # [k, p, q, n]
        wr_packed.append(np.ascontiguousarray(t.transpose(2, 1, 0, 3)))

    in_maps = []
    for c in range(NCORES):
        nh, ms = divmod(c, R)
        mc = slice(ms * MS, (ms + 1) * MS)
        ax = np.ascontiguousarray(
            xT[:, mc].reshape(KTX, P, MS).transpose(1, 0, 2)
        )
        az = np.ascontiguousarray(
            zcT[:, mc].reshape(KTX, P, MS).transpose(1, 0, 2)
        )
        in_maps.append(
            {"atx": ax, "atz": az, "wx": wx_packed[nh], "wr": wr_packed[nh]}
        )
    return in_maps


def kernel(x, v, z, z_out, w, wrec, _trace=False):
    x = np.ascontiguousarray(x, dtype=np.float32)
    v = np.ascontiguousarray(v, dtype=np.float32)
    z = np.ascontiguousarray(z, dtype=np.float32)
    z_out = np.ascontiguousarray(z_out, dtype=np.float32)
    w = np.ascontiguousarray(w, dtype=np.float32)
    wrec = np.ascontiguousarray(wrec, dtype=np.float32)

    z_out_new = BETA * z_out + z  # exact fp32; also the GEMM's second operand
    mu = np.float32(z_out_new.mean(dtype=np.float64))
    zc = z_out_new - mu

    nc = _get_program()
    in_maps = make_in_maps(x, zc, w, wrec)
    res = bass_utils.run_bass_kernel_spmd(
        nc, in_maps, core_ids=list(range(NCORES)), trace=_trace
    )

    S = np.empty((N, N), np.float32)
    for c, r in enumerate(res.results):
        nh, ms = divmod(c, R)
        blk = r["sout"].astype(np.float32).transpose(1, 0, 2).reshape(NH, MS)
        S[ms * MS : (ms + 1) * MS, nh * NH : (nh + 1) * NH] = blk.T

    # restore the rank-1 term removed by mean-centering: mu * colsum(wrec)
    colsum = wrec.sum(axis=0, dtype=np.float64).astype(np.float32)
    v_new = ALPHA * v - V_TH * z + S + mu * colsum[None, :]
    # batch row 0 drives the threshold mask: recompute it exactly on host
    # (fp64 matvecs) so GEMM quantization noise can never flip a mask bit.
    row0 = (
        ALPHA * v[0].astype(np.float64)
        - V_TH * z[0].astype(np.float64)
        + x[0].astype(np.float64) @ w.astype(np.float64)
        + z_out_new[0].astype(np.float64) @ wrec.astype(np.float64)
    )
    v_new[0, :] = row0.astype(np.float32)
    mask = (v_new[0, :] - V_TH) > 0.0
    z_new = np.ascontiguousarray(
        np.broadcast_to(mask[:, None].astype(np.float32), (N, N))
    )
    out = (v_new, z_new, z_out_new)
    if _trace:
        return out, res
    return out
